# revision 2
# baseline (speedup 1.0000x reference)
"""TRN2 Bass kernel for nn_MultiHeadEquivariant (dense_cnn).

Math (per batch sample X [F=256, G=256]):
  1. Ht[g, fp] = sum_f X[f, g] * Wb_p[f, fp]          (dense "before", fp32r)
  2. Z[fp, k]  = sum_g Ht[g, fp] * KtT[h(fp)][g, k]   (grouped conv, bf16,
     four heads concurrently via PE col-tiling tile_position=(0,32j))
  3. Out[o, k] = sum_fp Wa_p[fp, o] * Z[fp, k] + bias (dense "after", fp32r)

fp = h*32 + d is a host-side feature permutation making heads contiguous;
KtT[h] = kernel[h, pt].T is host-precomputed (params are tiny/replicated).
Data-parallel over batch: 128 samples per core on 8 cores.
"""
import numpy as np
import ml_dtypes
from contextlib import ExitStack

import concourse.bacc as bacc
import concourse.tile as tile
from concourse import mybir
from concourse.bass_utils import run_bass_kernel_spmd

F32 = mybir.dt.float32
F32R = mybir.dt.float32r
BF16 = mybir.dt.bfloat16

BATCH = 1024
F = 256
G = 256
HEADS = 8
N_CORES = 8
B_PER_CORE = BATCH // N_CORES  # 128
GROUP = 4                      # samples per input DMA / step-3 weight group


def build(n_samples=B_PER_CORE):
    """Build + compile the per-core Bacc program for n_samples samples."""
    assert n_samples % GROUP == 0
    nc = bacc.Bacc("TRN2", target_bir_lowering=False, debug=False)

    x_d = nc.dram_tensor("x", [n_samples, F, G], F32R, kind="ExternalInput").ap()
    p_d = nc.dram_tensor("params", [128, 1026], F32R, kind="ExternalInput").ap()
    pk_d = nc.dram_tensor("params_k", [128, 4096], BF16, kind="ExternalInput").ap()
    out_d = nc.dram_tensor("out", [n_samples, F, G], F32, kind="ExternalOutput").ap()

    with tile.TileContext(nc) as tc, ExitStack() as ctx:
        wpool = ctx.enter_context(tc.tile_pool(name="weights", bufs=1))
        xpool = ctx.enter_context(tc.tile_pool(name="x", bufs=4))
        hpool = ctx.enter_context(tc.tile_pool(name="ht", bufs=3))
        zpool = ctx.enter_context(tc.tile_pool(name="z", bufs=6))
        opool = ctx.enter_context(tc.tile_pool(name="o", bufs=3))
        psum = ctx.enter_context(tc.tile_pool(name="psum", bufs=1, space="PSUM"))

        # params: cols 0:512 wb_p [f%128, (fc, fp)]; 512:1024 wa_p [fp%128,
        # (fpc, o)]; 1024:1026 bias [o%128, oc].
        p_sb = wpool.tile([128, 1026], F32R)
        nc.sync.dma_start(p_sb[:], p_d[:])
        wb_sb = p_sb[:, 0:512]
        wa_sb = p_sb[:, 512:1024]
        bias_sb = p_sb[:, 1024:1026].bitcast(F32)
        # params_k: kt[h] bf16 [g%128, (gc, k)] at cols 512h : 512(h+1)
        pk_sb = wpool.tile([128, 4096], BF16)
        nc.sync.dma_start(pk_sb[:], pk_d[:])
        kt_sb = [pk_sb[:, 512 * h: 512 * (h + 1)] for h in range(HEADS)]

        for g0 in range(0, n_samples, GROUP):
            # one 1MiB DMA: x[g0:g0+4] -> [p, (s, fc, g)]
            x4 = xpool.tile([128, GROUP * 512], F32R, name="x4", tag="x4")
            nc.sync.dma_start(
                x4[:].rearrange("p (s c g) -> p s c g", s=GROUP, c=2),
                x_d[g0:g0 + GROUP].rearrange("s (c p) g -> p s c g", p=128),
            )

            z_sbs = []
            for si in range(GROUP):
                xo = si * 512
                # ---- step 1 (fp32r): Ht[g, fp] ----
                ht_ps = psum.tile([128, 512], F32, name="ht_ps", tag="ht_ps", bufs=2)
                for gc in range(2):
                    for fc in range(2):
                        nc.tensor.matmul(
                            ht_ps[:, gc * 256:(gc + 1) * 256],
                            lhsT=x4[:, xo + fc * 256 + gc * 128: xo + fc * 256 + gc * 128 + 128],
                            rhs=wb_sb[:, fc * 256:(fc + 1) * 256],
                            start=(fc == 0), stop=(fc == 1),
                        )
                ht_sb = hpool.tile([128, 512], BF16, name="ht_sb", tag="ht_sb")
                nc.vector.tensor_copy(ht_sb[:], ht_ps[:])

                # ---- step 2 (bf16, col-tiled 4 heads/strip-group) ----
                z_ps = psum.tile([128, 512], F32, name="z_ps", tag="z_ps", bufs=2)
                for fpc in range(2):
                    for j in range(4):
                        h = fpc * 4 + j
                        for gc in range(2):
                            nc.tensor.matmul(
                                z_ps[32 * j:32 * (j + 1), fpc * 256:(fpc + 1) * 256],
                                lhsT=ht_sb[:, gc * 256 + h * 32: gc * 256 + h * 32 + 32],
                                rhs=kt_sb[h][:, gc * 256:(gc + 1) * 256],
                                start=(gc == 0), stop=(gc == 1),
                                tile_position=(0, 32 * j),
                            )
                z_sb = zpool.tile([128, 512], F32R, name="z_sb", tag="z_sb")
                nc.vector.tensor_copy(z_sb[:], z_ps[:])
                z_sbs.append(z_sb)

            # ---- step 3 (fp32r), weight-stationary across the group ----
            out_pss = [
                psum.tile([128, 512], F32, name=f"out_ps{si}", tag=f"out_ps{si}")
                for si in range(GROUP)
            ]
            for oc in range(2):
                for fpc in range(2):
                    for si in range(GROUP):
                        nc.tensor.matmul(
                            out_pss[si][:, oc * 256:(oc + 1) * 256],
                            lhsT=wa_sb[:, fpc * 256 + oc * 128: fpc * 256 + oc * 128 + 128],
                            rhs=z_sbs[si][:, fpc * 256:(fpc + 1) * 256],
                            start=(fpc == 0), stop=(fpc == 1),
                        )

            # bias-add copies (ACT) + 2-sample output DMAs
            for s2 in range(0, GROUP, 2):
                o2 = opool.tile([128, 1024], F32, name="o2", tag="o2")
                for si in (s2, s2 + 1):
                    for oc in range(2):
                        nc.scalar.activation(
                            o2[:, (si - s2) * 512 + oc * 256: (si - s2) * 512 + (oc + 1) * 256],
                            out_pss[si][:, oc * 256:(oc + 1) * 256],
                            mybir.ActivationFunctionType.Identity,
                            bias=bias_sb[:, oc:oc + 1],
                        )
                nc.sync.dma_start(
                    out_d[g0 + s2: g0 + s2 + 2].rearrange("s (c p) k -> p s c k", p=128),
                    o2[:].rearrange("p (s c k) -> p s c k", s=2, c=2),
                )

    nc.compile()
    return nc


def _host_params(w_before, kern, w_after, bias, pt):
    """Pack host-side parameter arrays (permuted weights, expanded kernel)."""
    fp_to_f = np.array([(fp % 32) * 8 + (fp // 32) for fp in range(F)], np.int64)
    wb_p = np.ascontiguousarray(np.asarray(w_before, np.float32)[:, fp_to_f])
    wa_p = np.ascontiguousarray(np.asarray(w_after, np.float32)[fp_to_f, :])
    kern = np.asarray(kern, np.float32)
    pt = np.asarray(pt)
    # kfull[h, k, g] = kern[h, pt[k, g]];  kt[h] = kfull[h].T -> [g, k]
    kfull = kern[:, pt]                      # [H, G, G] (k, g)
    kt = np.ascontiguousarray(kfull.transpose(0, 2, 1))  # [H, g, k]
    bias = np.asarray(bias, np.float32).reshape(F, 1)

    P = np.zeros((128, 1026), np.float32)
    P[:, 0:512] = wb_p.reshape(2, 128, 256).transpose(1, 0, 2).reshape(128, 512)
    P[:, 512:1024] = wa_p.reshape(2, 128, 256).transpose(1, 0, 2).reshape(128, 512)
    P[:, 1024:1026] = bias.reshape(2, 128, 1).transpose(1, 0, 2).reshape(128, 2)
    PK = np.zeros((128, 4096), ml_dtypes.bfloat16)
    for h in range(HEADS):
        PK[:, 512 * h: 512 * (h + 1)] = (
            kt[h].reshape(2, 128, 256).transpose(1, 0, 2).reshape(128, 512)
            .astype(ml_dtypes.bfloat16))
    return P, PK


_nc_cache = {}


def _get_nc(n_samples=B_PER_CORE):
    if n_samples not in _nc_cache:
        _nc_cache[n_samples] = build(n_samples)
    return _nc_cache[n_samples]


def kernel(x, w_before, kernel, w_after, bias, pt):
    x = np.ascontiguousarray(np.asarray(x, np.float32))
    P, PK = _host_params(w_before, kernel, w_after, bias, pt)
    nc = _get_nc(B_PER_CORE)
    in_maps = [
        {"x": x[c * B_PER_CORE:(c + 1) * B_PER_CORE], "params": P, "params_k": PK}
        for c in range(N_CORES)
    ]
    res = run_bass_kernel_spmd(nc, in_maps, core_ids=list(range(N_CORES)))
    return np.concatenate([r["out"] for r in res.results], axis=0)
